# revision 20
# baseline (speedup 1.0000x reference)
"""Trainium2 Bass kernel for nn_Conv2dKan (KAN-style 3x3 conv, 64->128 ch).

Math: out[b,o,l] = sum_k silu(u)*w_b + sum_{n,k} H_n(u)*(c*w_s), with u =
unfold(x) (3x3, pad 1). Linear in the basis functions, so the Hermite basis
H_0..H_7 is re-expressed in the monomial basis {u, u^2, ..., u^7} with the
basis change folded into the weights on the host; silu itself is folded in
as a degree-7 least-squares polynomial fit over the actual input values.
Constant terms are a per-o bias added on the host after gather.

Device work per core (one batch item): x arrives pre-padded as a [64, 2500]
tile (fine-sliced DMAs so the first row tile lands early); chunk 0 of the
implicit GEMM runs K=64 matmuls straight off it while [x|x] / [1|x] tiles
are built on-chip (SBUF->SBUF DMA copies + memset) to feed the short
ACT/DVE/Pool chain producing the plane pairs [u^2|u^3], [u^4|u^5],
[u^6|u^7].  Implicit GEMM: chunk 0 (K=64) + 3 chunks (K=128) x 9 shifted
window taps x 5 row tiles, PSUM-accumulated in fp32, fp32r x fp32r.
Evacuation staggered per row tile (DVE PSUM->SBUF copy, then DMA out).

Sharding: batch 8 -> one image per NeuronCore, fully data parallel.
"""

import sys

if "/opt/trn_rl_repo" not in sys.path:
    sys.path.insert(0, "/opt/trn_rl_repo")

import numpy as np

import concourse.bacc as bacc
import concourse.bass as bass
import concourse.tile as tile
from concourse import mybir
from concourse.bass_utils import run_bass_kernel_spmd

# Problem constants (hardcoded per harness contract).
B = 8
C_IN = 64
C_OUT = 128
K = 3
N_BASIS = 8
H = W = 48
HP = WP = H + 2  # padded image
L = H * W
PADN = HP * WP  # 2500
NTAPS = K * K
NCHUNK = 4
ROW_TILES = (10, 10, 10, 10, 8)
N_WARM = 12

_CACHE = {}


def _build_program():
    nc = bacc.Bacc("TRN2", target_bir_lowering=False, debug=False, num_devices=1)
    f32 = mybir.dt.float32
    f32r = mybir.dt.float32r
    ACT = mybir.ActivationFunctionType

    xx_d = nc.dram_tensor("xx", [128, PADN], f32r, kind="ExternalInput").ap()
    w0_d = nc.dram_tensor("w0", [64, NTAPS * 128], f32r, kind="ExternalInput").ap()
    w_d = nc.dram_tensor("w", [128, 3 * NTAPS * 128], f32r, kind="ExternalInput").ap()
    o_d = nc.dram_tensor("out", [C_OUT, L], f32, kind="ExternalOutput").ap()

    # x DMA slices: boundaries aligned so row tile r (rows 10r..10r+R+1,
    # i.e. cols < (10r+R+2)*50) is covered by the first slices.
    XS = (625, 1250, 1875, PADN)
    CS = (0, 834, 1667, PADN)  # slice bounds for elementwise / copies

    with tile.TileContext(nc) as tc:
        with (
            tc.tile_pool(name="big", bufs=1) as wpool,
            tc.tile_pool(name="outs", bufs=3) as opool,
            tc.tile_pool(name="psum", bufs=1, space="PSUM") as ppool,
        ):
            x_sb = wpool.tile([128, PADN], f32r, tag="xx")        # [x | x]
            t2 = wpool.tile([128, PADN], f32, tag="t2")          # [- | s]
            t3 = wpool.tile([128, PADN], f32, tag="t3")          # [- | s2]
            t23 = wpool.tile([128, PADN], f32, tag="t23")        # [- | s3]
            c1 = wpool.tile([128, PADN], f32r, tag="c1")         # [s | us]
            c2 = wpool.tile([128, PADN], f32r, tag="c2")         # [s2 | us2]
            c3 = wpool.tile([128, PADN], f32r, tag="c3")         # [s3 | us3]
            w0_sb = wpool.tile([128, NTAPS * 128], f32r, tag="w0")
            w_sb = wpool.tile([128, 3 * NTAPS * 128], f32r)
            warm = wpool.tile([128, 256], f32r, tag="warm")

            x_f32 = x_sb.bitcast(f32)
            c1f = c1.bitcast(f32)
            c2f = c2.bitcast(f32)
            c3f = c3.bitcast(f32)

            # ---- input DMAs ----
            # DMA queues share ~358GB/s aggregate, so only the critical
            # early pieces (x + chunk-0 weights) are issued immediately;
            # w1/w2/w3 issue later, behind ops that wait on x (see below).
            # DVE issues no DMAs and is idle early: it zeroes the PE warm
            # tile and the chunk-0 upper weights.
            nc.vector.memset(warm.bitcast(f32)[:], 0.0)
            nc.vector.memset(w0_sb.bitcast(f32)[64:128, :], 0.0)
            # x: lower-half pieces on sync, upper-half pieces on gpsimd
            XP = (0, 625, 1250, 1875, PADN)
            for p in range(4):
                nc.sync.dma_start(out=x_sb[0:64, XP[p] : XP[p + 1]], in_=xx_d[0:64, XP[p] : XP[p + 1]])
            for p in range(4):
                nc.gpsimd.dma_start(out=x_sb[64:128, XP[p] : XP[p + 1]], in_=xx_d[64:128, XP[p] : XP[p + 1]])
            # scalar ring: chunk-0 weights in 3-tap pieces (lower half only;
            # the zero upper half is memset by DVE).
            WB = NTAPS * 128
            for p in range(3):
                nc.scalar.dma_start(
                    out=w0_sb[0:64, p * 384 : (p + 1) * 384], in_=w0_d[:, p * 384 : (p + 1) * 384]
                )
            HWB = WB // 2

            # ---- PE pre-warm while DMAs land (HAM/pstate ramp) ----
            warm_ps = ppool.tile([128, 256], f32, tag="warm_ps")
            for _ in range(N_WARM):
                nc.tensor.matmul(warm_ps[:], warm[:, 0:128], warm[:], start=True, stop=True)

            # ---- feature planes (half-partition ops; no [1|x] helper) ----
            # lower halves: s=x^2 in c1, s^2 in c2, s^3 in c3
            # upper halves: s,s^2,s^3 in t2/t3/t23, then *x -> c1/c2/c3
            LO = slice(0, 64)
            UP = slice(64, 128)
            for b in range(3):
                cs = slice(CS[b], CS[b + 1])
                nc.scalar.activation(c1[LO, cs], x_f32[LO, cs], ACT.Square)
                nc.scalar.activation(t2[UP, cs], x_f32[UP, cs], ACT.Square)
                if b == 0:
                    # w1 issues here: behind ACTs that waited on x, so its
                    # DMA doesn't steal bandwidth from the critical pieces
                    nc.scalar.dma_start(out=w_sb[:, 0:HWB], in_=w_d[:, 0:HWB])
                    nc.scalar.dma_start(out=w_sb[:, HWB:WB], in_=w_d[:, HWB:WB])
            for b in range(3):
                cs = slice(CS[b], CS[b + 1])
                nc.vector.tensor_mul(c1[UP, cs], t2[UP, cs], x_f32[UP, cs])
                nc.scalar.activation(c2[LO, cs], c1f[LO, cs], ACT.Square)
                nc.scalar.activation(t3[UP, cs], t2[UP, cs], ACT.Square)
            for b in range(3):
                cs = slice(CS[b], CS[b + 1])
                nc.vector.tensor_mul(c2[UP, cs], t3[UP, cs], x_f32[UP, cs])
                nc.gpsimd.tensor_mul(c3[LO, cs], c2f[LO, cs], c1f[LO, cs])
                if b == 0:
                    # w2 deferred likewise on the gpsimd ring
                    nc.gpsimd.dma_start(out=w_sb[:, 2 * HWB : 3 * HWB], in_=w_d[:, 2 * HWB : 3 * HWB])
                    nc.gpsimd.dma_start(out=w_sb[:, 3 * HWB : 4 * HWB], in_=w_d[:, 3 * HWB : 4 * HWB])
                nc.gpsimd.tensor_mul(t23[UP, cs], t3[UP, cs], t2[UP, cs])
                if b == 0:
                    # w3 deferred after w2
                    nc.gpsimd.dma_start(out=w_sb[:, 4 * HWB : 5 * HWB], in_=w_d[:, 4 * HWB : 5 * HWB])
                    nc.gpsimd.dma_start(out=w_sb[:, 5 * HWB : 6 * HWB], in_=w_d[:, 5 * HWB : 6 * HWB])
            for b in range(3):
                cs = slice(CS[b], CS[b + 1])
                nc.vector.tensor_mul(c3[UP, cs], t23[UP, cs], x_f32[UP, cs])

            # ---- implicit GEMM: chunk-outer, tile-mid, tap-inner ----
            x_im = x_sb.rearrange("c (h w) -> c h w", h=HP)
            chunk_ims = [t.rearrange("c (h w) -> c h w", h=HP) for t in (c1, c2, c3)]
            psums = []
            h0s = []
            h0 = 0
            for it, R in enumerate(ROW_TILES):
                psums.append(ppool.tile([128, R * W], f32, name=f"ps{h0}", tag=f"ps{it}"))
                h0s.append(h0)
                h0 += R
            out_rings = (nc.sync, nc.gpsimd, nc.sync, nc.gpsimd)

            # chunk 0: [x|x] tile, upper-half weights zero
            for it, R in enumerate(ROW_TILES):
                h0 = h0s[it]
                for t9 in range(NTAPS):
                    dh, dw = t9 // K - 1, t9 % K - 1
                    r0 = h0 + dh + 1
                    nc.tensor.matmul(
                        psums[it][:],
                        w0_sb[:, t9 * 128 : (t9 + 1) * 128],
                        x_im[:, r0 : r0 + R, dw + 1 : dw + 1 + W],
                        start=(t9 == 0),
                        stop=False,
                    )
            # chunks 1-3 (K=128), staggered per-tile evacuation on the last
            for jj, im in enumerate(chunk_ims):
                for it, R in enumerate(ROW_TILES):
                    h0 = h0s[it]
                    for t9 in range(NTAPS):
                        dh, dw = t9 // K - 1, t9 % K - 1
                        r0 = h0 + dh + 1
                        nc.tensor.matmul(
                            psums[it][:],
                            w_sb[:, (jj * NTAPS + t9) * 128 : (jj * NTAPS + t9 + 1) * 128],
                            im[:, r0 : r0 + R, dw + 1 : dw + 1 + W],
                            start=False,
                            stop=(jj == 2 and t9 == NTAPS - 1),
                        )
                    if jj == 2:
                        # staggered evacuation: DVE PSUM->SBUF, then DMA out
                        o_sb = opool.tile([C_OUT, R * W], f32, tag="osb")
                        if it < len(ROW_TILES) - 1:
                            nc.vector.tensor_copy(o_sb[:], psums[it][:])
                            out_rings[it].dma_start(
                                out=o_d[:, h0 * W : (h0 + R) * W], in_=o_sb[:]
                            )
                        else:
                            # last tile: halve so the final DMA starts sooner
                            hn = R * W // 2
                            for hh, eng in ((0, nc.sync), (1, nc.gpsimd)):
                                nc.vector.tensor_copy(
                                    o_sb[:, hh * hn : (hh + 1) * hn],
                                    psums[it][:, hh * hn : (hh + 1) * hn],
                                )
                                eng.dma_start(
                                    out=o_d[:, h0 * W + hh * hn : h0 * W + (hh + 1) * hn],
                                    in_=o_sb[:, hh * hn : (hh + 1) * hn],
                                )

    nc.compile()
    return nc


def _host_prep(x, w_b, w_s, c):
    """Fold Hermite->monomial basis change, w_s, and a degree-7 polynomial
    fit of silu into the weights (fp64 host math)."""
    wb = w_b[..., 0].astype(np.float64)          # (O, 576)
    cw = (c[..., 0] * w_s[None, ..., 0]).astype(np.float64)  # (N, O, 576)

    # monomial weights for planes u^1..u^7 (+ constant -> bias)
    wm = np.zeros((8, C_OUT, C_IN * NTAPS), np.float64)
    wm[1] = 2 * cw[1] - 12 * cw[3] + 120 * cw[5] - 1680 * cw[7]
    wm[2] = 2 * cw[2] - 48 * cw[4] + 720 * cw[6]
    wm[3] = 8 * cw[3] - 160 * cw[5] + 3360 * cw[7]
    wm[4] = 16 * cw[4] - 480 * cw[6]
    wm[5] = 32 * cw[5] - 1344 * cw[7]
    wm[6] = 64 * cw[6]
    wm[7] = 128 * cw[7]
    bias = (cw[0] - 2 * cw[2] + 12 * cw[4] - 120 * cw[6]).sum(axis=1)  # (O,)

    # degree-7 LS fit of silu over the actual input values (+ Chebyshev
    # nodes over the input range for tail control), folded into wm/bias
    xs = np.asarray(x, np.float64).ravel()
    m = np.abs(xs).max() * 1.02
    nodes = m * np.cos(np.pi * (np.arange(2000) + 0.5) / 2000)
    fitx = np.concatenate([xs[::37], nodes, nodes, nodes])
    A = np.vander(fitx, 8, increasing=True)
    coef, *_ = np.linalg.lstsq(A, fitx / (1 + np.exp(-fitx)), rcond=None)
    for f in range(1, 8):
        wm[f] += coef[f] * wb
    bias = bias + coef[0] * wb.sum(axis=1)

    # chunk 0 (plane u, K=64): [k=64, tap=9, o=128]
    cidx = np.arange(C_IN)
    w0 = np.zeros((64, NTAPS, C_OUT), np.float32)
    for t in range(NTAPS):
        w0[:, t, :] = wm[1][:, cidx * NTAPS + t].T.astype(np.float32)
    # chunks 1-3: [k_part=128, chunk=3, tap=9, o=128]
    # chunk j, k_part = 64*half + c_in -> plane u^{2j+2+half}
    wl = np.zeros((128, 3, NTAPS, C_OUT), np.float32)
    for j in range(3):
        for half in range(2):
            f = 2 * j + 2 + half
            for t in range(NTAPS):
                wl[64 * half : 64 * (half + 1), j, t, :] = (
                    wm[f][:, cidx * NTAPS + t].T.astype(np.float32)
                )
    return (
        w0.reshape(64, NTAPS * 128),
        wl.reshape(128, 3 * NTAPS * 128),
        bias.astype(np.float32),
    )


def _prep_in_maps(x, w_b, w_s, c):
    w0, wl, bias = _host_prep(x, w_b, w_s, c)
    xi = np.asarray(x, np.float32)
    xp = np.zeros((B, C_IN, HP, WP), np.float32)
    xp[:, :, 1 : 1 + H, 1 : 1 + W] = xi
    xp = xp.reshape(B, C_IN, PADN)
    in_maps = []
    for i in range(B):
        xx = np.concatenate([xp[i], xp[i]], axis=0)        # [x | x]
        in_maps.append({"xx": xx, "w0": w0, "w": wl})
    return in_maps, bias


def kernel(x, w_b, w_s, c):
    if "nc" not in _CACHE:
        _CACHE["nc"] = _build_program()
    nc = _CACHE["nc"]

    in_maps, bias = _prep_in_maps(x, w_b, w_s, c)
    res = run_bass_kernel_spmd(nc, in_maps, core_ids=list(range(B)))
    out = np.stack([res.results[i]["out"] for i in range(B)], axis=0)
    out += bias[None, :, None]
    return out.reshape(B, C_OUT, H, W)


# revision 22
# speedup vs baseline: 1.1876x; 1.1876x over previous
"""Trainium2 Bass kernel for nn_Conv2dKan (KAN-style 3x3 conv, 64->128 ch).

Math: out[b,o,l] = sum_k silu(u)*w_b + sum_{n,k} H_n(u)*(c*w_s), with u =
unfold(x) (3x3, pad 1). Linear in the basis functions, so the Hermite basis
H_0..H_7 is re-expressed in the monomial basis {u, u^2, ..., u^7} with the
basis change folded into the weights on the host; silu itself is folded in
as a degree-7 least-squares polynomial fit over the actual input values.
Constant terms are a per-o bias added on the host after gather.

Device work per core (one batch item): x arrives pre-padded as a [64, 2500]
tile (fine-sliced DMAs so the first row tile lands early); chunk 0 of the
implicit GEMM runs K=64 matmuls straight off it while [x|x] / [1|x] tiles
are built on-chip (SBUF->SBUF DMA copies + memset) to feed the short
ACT/DVE/Pool chain producing the plane pairs [u^2|u^3], [u^4|u^5],
[u^6|u^7].  Implicit GEMM: chunk 0 (K=64) + 3 chunks (K=128) x 9 shifted
window taps x 5 row tiles, PSUM-accumulated in fp32, fp32r x fp32r.
Evacuation staggered per row tile (DVE PSUM->SBUF copy, then DMA out).

Sharding: batch 8 -> one image per NeuronCore, fully data parallel.
"""

import sys

if "/opt/trn_rl_repo" not in sys.path:
    sys.path.insert(0, "/opt/trn_rl_repo")

import numpy as np

import concourse.bacc as bacc
import concourse.bass as bass
import concourse.tile as tile
from concourse import mybir
from concourse.bass_utils import run_bass_kernel_spmd

# Problem constants (hardcoded per harness contract).
B = 8
C_IN = 64
C_OUT = 128
K = 3
N_BASIS = 8
H = W = 48
HP = WP = H + 2  # padded image
L = H * W
PADN = HP * WP  # 2500
NTAPS = K * K
NCHUNK = 4
ROW_TILES = (10, 10, 10, 10, 8)
N_WARM = 12

_CACHE = {}


def _build_program():
    nc = bacc.Bacc("TRN2", target_bir_lowering=False, debug=False, num_devices=1)
    f32 = mybir.dt.float32
    f32r = mybir.dt.float32r
    ACT = mybir.ActivationFunctionType

    xx_d = nc.dram_tensor("xx", [128, PADN], f32r, kind="ExternalInput").ap()
    w0_d = nc.dram_tensor("w0", [64, NTAPS * 128], f32r, kind="ExternalInput").ap()
    w_d = nc.dram_tensor("w", [128, 3 * NTAPS * 128], f32r, kind="ExternalInput").ap()
    o_d = nc.dram_tensor("out", [C_OUT, L], f32, kind="ExternalOutput").ap()

    # x DMA slices: boundaries aligned so row tile r (rows 10r..10r+R+1,
    # i.e. cols < (10r+R+2)*50) is covered by the first slices.
    XS = (625, 1250, 1875, PADN)
    CS = (0, 834, 1667, PADN)  # slice bounds for elementwise / copies

    with tile.TileContext(nc) as tc:
        with (
            tc.tile_pool(name="big", bufs=1) as wpool,
            tc.tile_pool(name="outs", bufs=3) as opool,
            tc.tile_pool(name="psum", bufs=1, space="PSUM") as ppool,
        ):
            x_sb = wpool.tile([128, PADN], f32r, tag="xx")        # [x | x]
            t2 = wpool.tile([128, PADN], f32, tag="t2")          # [- | s]
            t3 = wpool.tile([128, PADN], f32, tag="t3")          # [- | s2]
            t23 = wpool.tile([128, PADN], f32, tag="t23")        # [- | s3]
            c1 = wpool.tile([128, PADN], f32r, tag="c1")         # [s | us]
            c2 = wpool.tile([128, PADN], f32r, tag="c2")         # [s2 | us2]
            c3 = wpool.tile([128, PADN], f32r, tag="c3")         # [s3 | us3]
            w0_sb = wpool.tile([128, NTAPS * 128], f32r, tag="w0")
            w_sb = wpool.tile([128, 3 * NTAPS * 128], f32r)
            warm = wpool.tile([128, 256], f32r, tag="warm")

            x_f32 = x_sb.bitcast(f32)
            c1f = c1.bitcast(f32)
            c2f = c2.bitcast(f32)
            c3f = c3.bitcast(f32)

            # ---- input DMAs (fine-sliced; each dma_start gets its own
            # hardware queue ~45GB/s, so slicing shortens the landing) ----
            # x lower half on sync, x upper half first on gpsimd: the two
            # halves of the first 625 cols stream in parallel.
            nc.sync.dma_start(out=x_sb[0:64, 0:625], in_=xx_d[0:64, 0:625])
            nc.gpsimd.dma_start(out=x_sb[64:128, 0:625], in_=xx_d[64:128, 0:625])
            for b in range(3):
                nc.sync.dma_start(out=x_sb[:, XS[b] : XS[b + 1]], in_=xx_d[:, XS[b] : XS[b + 1]])
            # scalar ring: chunk-0 weights in 3-tap pieces (lower half only;
            # the zero upper half is memset on-device), then chunk 1 (2 pcs).
            WB = NTAPS * 128
            for p in range(3):
                nc.scalar.dma_start(
                    out=w0_sb[0:64, p * 384 : (p + 1) * 384], in_=w0_d[:, p * 384 : (p + 1) * 384]
                )
            HWB = WB // 2
            nc.scalar.dma_start(out=w_sb[:, 0:HWB], in_=w_d[:, 0:HWB])
            nc.scalar.dma_start(out=w_sb[:, HWB:WB], in_=w_d[:, HWB:WB])
            # sync ring continues: w chunks 2-3 in halves
            for p in range(2, 6):
                nc.sync.dma_start(
                    out=w_sb[:, p * HWB : (p + 1) * HWB], in_=w_d[:, p * HWB : (p + 1) * HWB]
                )

            # gpsimd ring: PE warm tile + chunk-0 upper weights = 0
            nc.gpsimd.memset(warm.bitcast(f32)[:], 0.0)
            nc.gpsimd.memset(w0_sb.bitcast(f32)[64:128, :], 0.0)

            # ---- PE pre-warm while DMAs land (HAM/pstate ramp) ----
            warm_ps = ppool.tile([128, 256], f32, tag="warm_ps")
            for _ in range(N_WARM):
                nc.tensor.matmul(warm_ps[:], warm[:, 0:128], warm[:], start=True, stop=True)

            # ---- feature planes (half-partition ops; no [1|x] helper) ----
            # lower halves: s=x^2 in c1, s^2 in c2, s^3 in c3
            # upper halves: s,s^2,s^3 in t2/t3/t23, then *x -> c1/c2/c3
            LO = slice(0, 64)
            UP = slice(64, 128)
            for b in range(3):
                cs = slice(CS[b], CS[b + 1])
                nc.scalar.activation(c1[LO, cs], x_f32[LO, cs], ACT.Square)
                nc.scalar.activation(t2[UP, cs], x_f32[UP, cs], ACT.Square)
            for b in range(3):
                cs = slice(CS[b], CS[b + 1])
                nc.vector.tensor_mul(c1[UP, cs], t2[UP, cs], x_f32[UP, cs])
                nc.scalar.activation(c2[LO, cs], c1f[LO, cs], ACT.Square)
                nc.scalar.activation(t3[UP, cs], t2[UP, cs], ACT.Square)
            for b in range(3):
                cs = slice(CS[b], CS[b + 1])
                nc.vector.tensor_mul(c2[UP, cs], t3[UP, cs], x_f32[UP, cs])
                nc.gpsimd.tensor_mul(c3[LO, cs], c2f[LO, cs], c1f[LO, cs])
                nc.gpsimd.tensor_mul(t23[UP, cs], t3[UP, cs], t2[UP, cs])
            for b in range(3):
                cs = slice(CS[b], CS[b + 1])
                nc.vector.tensor_mul(c3[UP, cs], t23[UP, cs], x_f32[UP, cs])

            # ---- implicit GEMM: chunk-outer, tile-mid, tap-inner ----
            x_im = x_sb.rearrange("c (h w) -> c h w", h=HP)
            chunk_ims = [t.rearrange("c (h w) -> c h w", h=HP) for t in (c1, c2, c3)]
            psums = []
            h0s = []
            h0 = 0
            for it, R in enumerate(ROW_TILES):
                psums.append(ppool.tile([128, R * W], f32, name=f"ps{h0}", tag=f"ps{it}"))
                h0s.append(h0)
                h0 += R
            out_rings = (nc.sync, nc.gpsimd, nc.sync, nc.gpsimd)

            # chunk 0: [x|x] tile, upper-half weights zero
            for it, R in enumerate(ROW_TILES):
                h0 = h0s[it]
                for t9 in range(NTAPS):
                    dh, dw = t9 // K - 1, t9 % K - 1
                    r0 = h0 + dh + 1
                    nc.tensor.matmul(
                        psums[it][:],
                        w0_sb[:, t9 * 128 : (t9 + 1) * 128],
                        x_im[:, r0 : r0 + R, dw + 1 : dw + 1 + W],
                        start=(t9 == 0),
                        stop=False,
                    )
            # chunks 1-3 (K=128), staggered per-tile evacuation on the last
            for jj, im in enumerate(chunk_ims):
                for it, R in enumerate(ROW_TILES):
                    h0 = h0s[it]
                    for t9 in range(NTAPS):
                        dh, dw = t9 // K - 1, t9 % K - 1
                        r0 = h0 + dh + 1
                        nc.tensor.matmul(
                            psums[it][:],
                            w_sb[:, (jj * NTAPS + t9) * 128 : (jj * NTAPS + t9 + 1) * 128],
                            im[:, r0 : r0 + R, dw + 1 : dw + 1 + W],
                            start=False,
                            stop=(jj == 2 and t9 == NTAPS - 1),
                        )
                    if jj == 2:
                        # staggered evacuation: DVE PSUM->SBUF, then DMA out
                        o_sb = opool.tile([C_OUT, R * W], f32, tag="osb")
                        if it < len(ROW_TILES) - 1:
                            nc.vector.tensor_copy(o_sb[:], psums[it][:])
                            out_rings[it].dma_start(
                                out=o_d[:, h0 * W : (h0 + R) * W], in_=o_sb[:]
                            )
                        else:
                            # last tile: halve so the final DMA starts sooner
                            hn = R * W // 2
                            for hh, eng in ((0, nc.sync), (1, nc.gpsimd)):
                                nc.vector.tensor_copy(
                                    o_sb[:, hh * hn : (hh + 1) * hn],
                                    psums[it][:, hh * hn : (hh + 1) * hn],
                                )
                                eng.dma_start(
                                    out=o_d[:, h0 * W + hh * hn : h0 * W + (hh + 1) * hn],
                                    in_=o_sb[:, hh * hn : (hh + 1) * hn],
                                )

    nc.compile()
    return nc


def _host_prep(x, w_b, w_s, c):
    """Fold Hermite->monomial basis change, w_s, and a degree-7 polynomial
    fit of silu into the weights (fp64 host math)."""
    wb = w_b[..., 0].astype(np.float64)          # (O, 576)
    cw = (c[..., 0] * w_s[None, ..., 0]).astype(np.float64)  # (N, O, 576)

    # monomial weights for planes u^1..u^7 (+ constant -> bias)
    wm = np.zeros((8, C_OUT, C_IN * NTAPS), np.float64)
    wm[1] = 2 * cw[1] - 12 * cw[3] + 120 * cw[5] - 1680 * cw[7]
    wm[2] = 2 * cw[2] - 48 * cw[4] + 720 * cw[6]
    wm[3] = 8 * cw[3] - 160 * cw[5] + 3360 * cw[7]
    wm[4] = 16 * cw[4] - 480 * cw[6]
    wm[5] = 32 * cw[5] - 1344 * cw[7]
    wm[6] = 64 * cw[6]
    wm[7] = 128 * cw[7]
    bias = (cw[0] - 2 * cw[2] + 12 * cw[4] - 120 * cw[6]).sum(axis=1)  # (O,)

    # degree-7 LS fit of silu over the actual input values (+ Chebyshev
    # nodes over the input range for tail control), folded into wm/bias
    xs = np.asarray(x, np.float64).ravel()
    m = np.abs(xs).max() * 1.02
    nodes = m * np.cos(np.pi * (np.arange(2000) + 0.5) / 2000)
    fitx = np.concatenate([xs[::37], nodes, nodes, nodes])
    A = np.vander(fitx, 8, increasing=True)
    coef, *_ = np.linalg.lstsq(A, fitx / (1 + np.exp(-fitx)), rcond=None)
    for f in range(1, 8):
        wm[f] += coef[f] * wb
    bias = bias + coef[0] * wb.sum(axis=1)

    # chunk 0 (plane u, K=64): [k=64, tap=9, o=128]
    cidx = np.arange(C_IN)
    w0 = np.zeros((64, NTAPS, C_OUT), np.float32)
    for t in range(NTAPS):
        w0[:, t, :] = wm[1][:, cidx * NTAPS + t].T.astype(np.float32)
    # chunks 1-3: [k_part=128, chunk=3, tap=9, o=128]
    # chunk j, k_part = 64*half + c_in -> plane u^{2j+2+half}
    wl = np.zeros((128, 3, NTAPS, C_OUT), np.float32)
    for j in range(3):
        for half in range(2):
            f = 2 * j + 2 + half
            for t in range(NTAPS):
                wl[64 * half : 64 * (half + 1), j, t, :] = (
                    wm[f][:, cidx * NTAPS + t].T.astype(np.float32)
                )
    return (
        w0.reshape(64, NTAPS * 128),
        wl.reshape(128, 3 * NTAPS * 128),
        bias.astype(np.float32),
    )


def _prep_in_maps(x, w_b, w_s, c):
    w0, wl, bias = _host_prep(x, w_b, w_s, c)
    xi = np.asarray(x, np.float32)
    xp = np.zeros((B, C_IN, HP, WP), np.float32)
    xp[:, :, 1 : 1 + H, 1 : 1 + W] = xi
    xp = xp.reshape(B, C_IN, PADN)
    in_maps = []
    for i in range(B):
        xx = np.concatenate([xp[i], xp[i]], axis=0)        # [x | x]
        in_maps.append({"xx": xx, "w0": w0, "w": wl})
    return in_maps, bias


def kernel(x, w_b, w_s, c):
    if "nc" not in _CACHE:
        _CACHE["nc"] = _build_program()
    nc = _CACHE["nc"]

    in_maps, bias = _prep_in_maps(x, w_b, w_s, c)
    res = run_bass_kernel_spmd(nc, in_maps, core_ids=list(range(B)))
    out = np.stack([res.results[i]["out"] for i in range(B)], axis=0)
    out += bias[None, :, None]
    return out.reshape(B, C_OUT, H, W)


# revision 26
# speedup vs baseline: 1.2627x; 1.0632x over previous
"""Trainium2 Bass kernel for nn_Conv2dKan (KAN-style 3x3 conv, 64->128 ch).

Math: out[b,o,l] = sum_k silu(u)*w_b + sum_{n,k} H_n(u)*(c*w_s), with u =
unfold(x) (3x3, pad 1). Linear in the basis functions, so the Hermite basis
H_0..H_7 is re-expressed in the monomial basis {u, u^2, ..., u^7} with the
basis change folded into the weights on the host; silu itself is folded in
as a degree-7 least-squares polynomial fit over the actual input values.
Constant terms are a per-o bias added on the host after gather.

Device work per core (one batch item): the [x|x] input tile's upper half is
squared IN PLACE so it becomes implicit-GEMM chunk A = [x|x^2] with no
copies; chunks B=[x^3|x^4] and C=[x^5|x^6] come from a short ACT/DVE/Pool
chain; chunk D pairs the lonely 7th plane with its own column-shifted copy
[x^7 | x^7>>1col], which lets one matmul pass cover two filter taps - D
needs only 6 passes for its 9 taps (33 passes total instead of 36).
All matmuls are K=128 fp32r x fp32r, PSUM-accumulated per row tile (9+9+9+6
passes), staggered DVE evacuation + DMA out.  Input DMAs are fine-sliced
across queues so the first row tile and chunk-A weights land first.

Sharding: batch 8 -> one image per NeuronCore, fully data parallel.
"""

import sys

if "/opt/trn_rl_repo" not in sys.path:
    sys.path.insert(0, "/opt/trn_rl_repo")

import numpy as np

import concourse.bacc as bacc
import concourse.bass as bass
import concourse.tile as tile
from concourse import mybir
from concourse.bass_utils import run_bass_kernel_spmd

# Problem constants (hardcoded per harness contract).
B = 8
C_IN = 64
C_OUT = 128
K = 3
N_BASIS = 8
H = W = 48
HP = WP = H + 2  # padded image
L = H * W
PADN = HP * WP  # 2500
NTAPS = K * K
ROW_TILES = (10, 10, 10, 10, 8)
N_WARM = 12
ND = 6  # chunk-D passes (tap pairs)

_CACHE = {}


def _build_program():
    nc = bacc.Bacc("TRN2", target_bir_lowering=False, debug=False, num_devices=1)
    f32 = mybir.dt.float32
    f32r = mybir.dt.float32r
    ACT = mybir.ActivationFunctionType

    xx_d = nc.dram_tensor("xx", [128, PADN], f32r, kind="ExternalInput").ap()
    xu_d = nc.dram_tensor("xu", [64, PADN], f32, kind="ExternalInput").ap()
    wa_d = nc.dram_tensor("wa", [128, NTAPS * 128], f32r, kind="ExternalInput").ap()
    w_d = nc.dram_tensor("w", [128, (2 * NTAPS + ND) * 128], f32r, kind="ExternalInput").ap()
    o_d = nc.dram_tensor("out", [C_OUT, L], f32, kind="ExternalOutput").ap()

    XS = (625, 1250, 1875, PADN)
    CS = (0, 834, 1667, PADN)  # slice bounds for elementwise ops

    with tile.TileContext(nc) as tc:
        with (
            tc.tile_pool(name="big", bufs=1) as wpool,
            tc.tile_pool(name="outs", bufs=3) as opool,
            tc.tile_pool(name="psum", bufs=1, space="PSUM") as ppool,
        ):
            x_sb = wpool.tile([128, PADN], f32r, tag="xx")       # [x|x] -> A=[x|x^2]
            xu = wpool.tile([128, PADN], f32, tag="xu")          # plain x at partitions 64-127
            t2 = wpool.tile([128, PADN], f32, tag="t2")          # [s | -]
            bt = wpool.tile([128, PADN], f32r, tag="bt")         # B = [x^3|x^4]
            ct = wpool.tile([128, PADN], f32r, tag="ct")         # C = [x^5|x^6]
            dt_ = wpool.tile([128, PADN], f32r, tag="dt")        # D = [x^7|x^7>>1]
            wa_sb = wpool.tile([128, NTAPS * 128], f32r, tag="wa")
            w_sb = wpool.tile([128, (2 * NTAPS + ND) * 128], f32r)
            warm = wpool.tile([128, 256], f32r, tag="warm")

            x_f32 = x_sb.bitcast(f32)
            btf = bt.bitcast(f32)
            ctf = ct.bitcast(f32)

            LO = slice(0, 64)
            UP = slice(64, 128)

            # ---- input DMAs (fine-sliced across queues) ----
            # x: lower half on sync, upper half first piece on gpsimd
            nc.sync.dma_start(out=x_sb[0:64, 0:625], in_=xx_d[0:64, 0:625])
            nc.gpsimd.dma_start(out=x_sb[64:128, 0:625], in_=xx_d[64:128, 0:625])
            for b in range(3):
                nc.sync.dma_start(out=x_sb[:, XS[b] : XS[b + 1]], in_=xx_d[:, XS[b] : XS[b + 1]])
            # chunk-A weights in 3-tap pieces on scalar (these gate mm0)
            for p in range(3):
                nc.scalar.dma_start(
                    out=wa_sb[:, p * 384 : (p + 1) * 384], in_=wa_d[:, p * 384 : (p + 1) * 384]
                )
            # chunk B on scalar, C/D + plain upper x on sync (later need)
            WB = NTAPS * 128
            HWB = WB // 2
            nc.scalar.dma_start(out=w_sb[:, 0:HWB], in_=w_d[:, 0:HWB])
            nc.scalar.dma_start(out=w_sb[:, HWB:WB], in_=w_d[:, HWB:WB])
            for p in range(2, 4):
                nc.sync.dma_start(
                    out=w_sb[:, p * HWB : (p + 1) * HWB], in_=w_d[:, p * HWB : (p + 1) * HWB]
                )
            nc.sync.dma_start(
                out=w_sb[:, 4 * HWB : (2 * NTAPS + ND) * 128], in_=w_d[:, 4 * HWB : (2 * NTAPS + ND) * 128]
            )
            nc.sync.dma_start(out=xu[64:128, 0:1250], in_=xu_d[:, 0:1250])
            nc.sync.dma_start(out=xu[64:128, 1250:PADN], in_=xu_d[:, 1250:PADN])

            # gpsimd ring: PE warm tile + chunk-D shifted upper's last col
            nc.gpsimd.memset(warm.bitcast(f32)[:], 0.0)
            nc.gpsimd.memset(dt_.bitcast(f32)[UP, PADN - 1 : PADN], 0.0)

            # ---- PE pre-warm while DMAs land (HAM/pstate ramp) ----
            warm_ps = ppool.tile([128, 256], f32, tag="warm_ps")
            for _ in range(N_WARM):
                nc.tensor.matmul(warm_ps[:], warm[:, 0:128], warm[:], start=True, stop=True)

            # ---- feature planes ----
            # upper-half square IN PLACE, sliced to follow the x pieces so
            # chunk A's first row tile is ready as soon as possible
            XQ = (0, 625, 1250, 1875, PADN)
            for q in range(4):
                xs_ = slice(XQ[q], XQ[q + 1])
                nc.scalar.activation(x_sb[UP, xs_], x_f32[UP, xs_], ACT.Square)
            # lower: s = x^2; B/C/D lowers by repeated multiply (odd powers)
            for b in range(3):
                cs = slice(CS[b], CS[b + 1])
                nc.scalar.activation(t2[LO, cs], x_f32[LO, cs], ACT.Square)
                nc.vector.tensor_mul(bt[LO, cs], t2[LO, cs], x_f32[LO, cs])      # x^3
            for b in range(3):
                cs = slice(CS[b], CS[b + 1])
                nc.scalar.activation(bt[UP, cs], x_f32[UP, cs], ACT.Square)      # x^4=(x^2)^2
                nc.gpsimd.tensor_mul(ct[LO, cs], btf[LO, cs], t2[LO, cs])        # x^5
            for b in range(3):
                cs = slice(CS[b], CS[b + 1])
                nc.vector.tensor_mul(ct[UP, cs], btf[UP, cs], x_f32[UP, cs])     # x^6=x^4*x^2
                nc.gpsimd.tensor_mul(dt_[LO, cs], ctf[LO, cs], t2[LO, cs])       # x^7
            # D upper = x^7 shifted left one col: x^6[c+1] * x[c+1]
            for b in range(3):
                c0, c1_ = CS[b], CS[b + 1]
                if c1_ == PADN:
                    c1_ = PADN - 1  # last col memset to 0 on gpsimd above
                nc.vector.tensor_mul(
                    dt_[UP, c0:c1_], ctf[UP, c0 + 1 : c1_ + 1], xu[UP, c0 + 1 : c1_ + 1]
                )

            # ---- implicit GEMM: chunk-outer, tile-mid, pass-inner ----
            ims = [t.rearrange("c (h w) -> c h w", h=HP) for t in (x_sb, bt, ct, dt_)]
            psums = []
            h0s = []
            h0 = 0
            for it, R in enumerate(ROW_TILES):
                psums.append(ppool.tile([128, R * W], f32, name=f"ps{h0}", tag=f"ps{it}"))
                h0s.append(h0)
                h0 += R
            out_rings = (nc.sync, nc.gpsimd, nc.sync, nc.gpsimd)

            # chunks A, B, C: 9 taps each
            for j in range(3):
                im = ims[j]
                for it, R in enumerate(ROW_TILES):
                    h0 = h0s[it]
                    for t9 in range(NTAPS):
                        dh, dw = t9 // K - 1, t9 % K - 1
                        r0 = h0 + dh + 1
                        lhsT = (
                            wa_sb[:, t9 * 128 : (t9 + 1) * 128]
                            if j == 0
                            else w_sb[:, ((j - 1) * NTAPS + t9) * 128 : ((j - 1) * NTAPS + t9 + 1) * 128]
                        )
                        nc.tensor.matmul(
                            psums[it][:],
                            lhsT,
                            im[:, r0 : r0 + R, dw + 1 : dw + 1 + W],
                            start=(j == 0 and t9 == 0),
                            stop=False,
                        )
            # chunk D: 6 passes (dh x col-offset o in {0,1}); lower covers
            # tap (dh,o-1), shifted upper covers tap (dh,o)
            imd = ims[3]
            for it, R in enumerate(ROW_TILES):
                h0 = h0s[it]
                for p in range(ND):
                    dh, o = p // 2 - 1, p % 2
                    r0 = h0 + dh + 1
                    nc.tensor.matmul(
                        psums[it][:],
                        w_sb[:, (2 * NTAPS + p) * 128 : (2 * NTAPS + p + 1) * 128],
                        imd[:, r0 : r0 + R, o : o + W],
                        start=False,
                        stop=(p == ND - 1),
                    )
                # staggered evacuation: DVE PSUM->SBUF, then DMA out
                o_sb = opool.tile([C_OUT, R * W], f32, tag="osb")
                if it < len(ROW_TILES) - 1:
                    nc.vector.tensor_copy(o_sb[:], psums[it][:])
                    out_rings[it].dma_start(out=o_d[:, h0 * W : (h0 + R) * W], in_=o_sb[:])
                else:
                    # last tile: halve so the final DMA starts sooner
                    hn = R * W // 2
                    for hh, eng in ((0, nc.sync), (1, nc.gpsimd)):
                        nc.vector.tensor_copy(
                            o_sb[:, hh * hn : (hh + 1) * hn],
                            psums[it][:, hh * hn : (hh + 1) * hn],
                        )
                        eng.dma_start(
                            out=o_d[:, h0 * W + hh * hn : h0 * W + (hh + 1) * hn],
                            in_=o_sb[:, hh * hn : (hh + 1) * hn],
                        )

    nc.compile()
    return nc


def _host_prep(x, w_b, w_s, c):
    """Fold Hermite->monomial basis change, w_s, and a degree-7 polynomial
    fit of silu into the weights (fp64 host math)."""
    wb = w_b[..., 0].astype(np.float64)          # (O, 576)
    cw = (c[..., 0] * w_s[None, ..., 0]).astype(np.float64)  # (N, O, 576)

    # monomial weights for planes u^1..u^7 (+ constant -> bias)
    wm = np.zeros((8, C_OUT, C_IN * NTAPS), np.float64)
    wm[1] = 2 * cw[1] - 12 * cw[3] + 120 * cw[5] - 1680 * cw[7]
    wm[2] = 2 * cw[2] - 48 * cw[4] + 720 * cw[6]
    wm[3] = 8 * cw[3] - 160 * cw[5] + 3360 * cw[7]
    wm[4] = 16 * cw[4] - 480 * cw[6]
    wm[5] = 32 * cw[5] - 1344 * cw[7]
    wm[6] = 64 * cw[6]
    wm[7] = 128 * cw[7]
    bias = (cw[0] - 2 * cw[2] + 12 * cw[4] - 120 * cw[6]).sum(axis=1)  # (O,)

    # degree-7 LS fit of silu over the actual input values (+ Chebyshev
    # nodes over the input range for tail control), folded into wm/bias
    xs = np.asarray(x, np.float64).ravel()
    m = np.abs(xs).max() * 1.02
    nodes = m * np.cos(np.pi * (np.arange(2000) + 0.5) / 2000)
    fitx = np.concatenate([xs[::37], nodes, nodes, nodes])
    A = np.vander(fitx, 8, increasing=True)
    coef, *_ = np.linalg.lstsq(A, fitx / (1 + np.exp(-fitx)), rcond=None)
    for f in range(1, 8):
        wm[f] += coef[f] * wb
    bias = bias + coef[0] * wb.sum(axis=1)

    cidx = np.arange(C_IN)

    def tapw(f, t):
        # [64, 128] weight block: plane u^f, tap t
        return wm[f][:, cidx * NTAPS + t].T.astype(np.float32)

    # chunk A = [x | x^2]: 9 taps
    wa = np.zeros((128, NTAPS, C_OUT), np.float32)
    for t in range(NTAPS):
        wa[0:64, t] = tapw(1, t)
        wa[64:128, t] = tapw(2, t)
    # chunks B = [x^3|x^4], C = [x^5|x^6]: 9 taps each
    wl = np.zeros((128, 2 * NTAPS + ND, C_OUT), np.float32)
    for t in range(NTAPS):
        wl[0:64, t] = tapw(3, t)
        wl[64:128, t] = tapw(4, t)
        wl[0:64, NTAPS + t] = tapw(5, t)
        wl[64:128, NTAPS + t] = tapw(6, t)
    # chunk D = [x^7 | x^7>>1]: 6 passes (dh, o); lower = tap (dh, o-1),
    # upper = tap (dh, 1) on o==1 passes only
    for p in range(ND):
        dh, o = p // 2 - 1, p % 2
        t_lo = (dh + 1) * K + (o - 1 + 1)
        wl[0:64, 2 * NTAPS + p] = tapw(7, t_lo)
        if o == 1:
            wl[64:128, 2 * NTAPS + p] = tapw(7, (dh + 1) * K + 2)
    return (
        wa.reshape(128, NTAPS * 128),
        wl.reshape(128, (2 * NTAPS + ND) * 128),
        bias.astype(np.float32),
    )


def _prep_in_maps(x, w_b, w_s, c):
    wa, wl, bias = _host_prep(x, w_b, w_s, c)
    xi = np.asarray(x, np.float32)
    xp = np.zeros((B, C_IN, HP, WP), np.float32)
    xp[:, :, 1 : 1 + H, 1 : 1 + W] = xi
    xp = xp.reshape(B, C_IN, PADN)
    in_maps = []
    for i in range(B):
        xx = np.concatenate([xp[i], xp[i]], axis=0)        # [x | x]
        in_maps.append({"xx": xx, "xu": xp[i], "wa": wa, "w": wl})
    return in_maps, bias


def kernel(x, w_b, w_s, c):
    if "nc" not in _CACHE:
        _CACHE["nc"] = _build_program()
    nc = _CACHE["nc"]

    in_maps, bias = _prep_in_maps(x, w_b, w_s, c)
    res = run_bass_kernel_spmd(nc, in_maps, core_ids=list(range(B)))
    out = np.stack([res.results[i]["out"] for i in range(B)], axis=0)
    out += bias[None, :, None]
    return out.reshape(B, C_OUT, H, W)
